# revision 1
# baseline (speedup 1.0000x reference)
"""PointGRN (segment_reduce) Trainium2 Bass kernel.

Computation (per segment b, channel c over points feat [N, 64] f32):
    sumsq[b,c]  = sum_{n in seg b} feat[n,c]^2
    r[b,c]      = sqrt(sumsq[b,c])
    rn[b,c]     = r[b,c] / (mean_c r[b,:] + 1e-6)
    out[n,c]    = feat[n,c] * (1 + gamma[c]*rn[b,c]) + beta[c]

Sharding: data-parallel over segments — host reads `offset` and gives each
of the 8 cores one whole segment (padded with zero rows to a 128-row
multiple).  No device-side searchsorted and no collectives needed.

Device kernel (per core), DMA-bound at ~330 GB/s/core:
    pass 1: stream [128 x k*64] f32 tiles (k=32 plus one ragged tail);
            ACT squares into bf16; PE ones-matmul reduces partitions,
            accumulating into 4 PSUM rows.  The first RES tiles stay
            resident in SBUF.
    combine: tiny [1,64] vector math (sqrt + Newton step, mean, scale),
            broadcast scale/beta to [128,128] via a K=1 matmul.
    pass 2: resident tiles are rescaled in place (no reload); the rest are
            re-streamed; y = x*s + beta in place; store.  Loads ride the
            SP HWDGE ring, stores the ACT ring (~332 GB/s combined vs
            ~305 single-ring); +beta alternates DVE/GPSIMD so no engine
            becomes the pass-2 critical path.
"""

import numpy as np

import concourse.bacc as bacc
import concourse.bass as bass
import concourse.mybir as mybir
import concourse.tile as tile
from concourse.bass_utils import run_bass_kernel_spmd

EPS = 1e-06
N_CORES = 8
P = 128          # SBUF partitions
C = 64           # channels
K = 32           # row-groups per partition per full tile
F = K * C        # full-tile free dim (2048 f32 = 8KB/partition)
TILE_ROWS = P * K  # 4096 rows per full tile
MM_N = 512       # matmul moving free-dim chunk
NCHUNK = F // MM_N
RES = 20         # full tiles kept resident in SBUF between the two passes

_AFT = mybir.ActivationFunctionType
_ALU = mybir.AluOpType

_program_cache: dict[tuple, bass.Bass] = {}


def _tile_rows(r_pad):
    """Split r_pad rows into full [128 x K] tiles plus one ragged tail tile."""
    pchunks = r_pad // P
    nt_full = pchunks // K
    k_tail = pchunks % K
    ks = [K] * nt_full + ([k_tail] if k_tail else [])
    return ks


def _build_program(
    r_pad: int,
    repeats: int = 1,
    res: int = RES,
    add_eng: str = "gpsimd",
    bufs_x: int = 4,
) -> bass.Bass:
    """One-core Bass program for a shard of r_pad rows (r_pad % 128 == 0).

    `repeats` re-runs the whole computation body that many times (timing
    only: the wall-clock slope over repeats isolates kernel time from the
    ~80-100ms flat dispatch overhead of this axon environment).
    """
    from contextlib import ExitStack

    ks = _tile_rows(r_pad)
    nt = len(ks)
    res = min(res, sum(1 for k in ks if k == K))
    nc = bacc.Bacc()

    feat = nc.declare_dram_parameter("feat", [r_pad, C], mybir.dt.float32, isOutput=False)
    gamma = nc.declare_dram_parameter("gamma", [1, C], mybir.dt.float32, isOutput=False)
    beta = nc.declare_dram_parameter("beta", [1, C], mybir.dt.float32, isOutput=False)
    out = nc.declare_dram_parameter("out", [r_pad, C], mybir.dt.float32, isOutput=True)

    row0 = [0] * nt
    for t in range(1, nt):
        row0[t] = row0[t - 1] + P * ks[t - 1]

    def feat_view(t):
        r0 = row0[t]
        return feat[r0 : r0 + P * ks[t], :].rearrange("(p k) c -> p (k c)", k=ks[t])

    def out_view(t):
        r0 = row0[t]
        return out[r0 : r0 + P * ks[t], :].rearrange("(p k) c -> p (k c)", k=ks[t])

    with tile.TileContext(nc) as tc, ExitStack() as ctx:
        const = ctx.enter_context(tc.tile_pool(name="const", bufs=1))
        inp = ctx.enter_context(tc.tile_pool(name="inp", bufs=bufs_x))
        resp = ctx.enter_context(tc.tile_pool(name="resp", bufs=1))
        sqp = ctx.enter_context(tc.tile_pool(name="sqp", bufs=2))
        psum = ctx.enter_context(tc.tile_pool(name="psum", bufs=1, space="PSUM"))
        small = ctx.enter_context(tc.tile_pool(name="small", bufs=1))
        adder = getattr(nc, add_eng)

        ones_col = const.tile([P, 1], mybir.dt.bfloat16, name="ones_col", tag="ones_col")
        nc.vector.memset(ones_col, 1.0)
        ones_row = const.tile([1, P], mybir.dt.float32, name="ones_row", tag="ones_row")
        nc.vector.memset(ones_row, 1.0)

        # chunks actually written, and the last tile writing each (stop flag)
        nchunks = (max(ks) * C + MM_N - 1) // MM_N
        last_t_for_chunk = [0] * nchunks
        for t in range(nt):
            for j in range((ks[t] * C + MM_N - 1) // MM_N):
                last_t_for_chunk[j] = t

        for _rep in range(repeats):
            # --- pass 1: sum of squares ----------------------------------
            acc = [
                psum.tile([1, MM_N], mybir.dt.float32, name=f"acc{j}", tag=f"acc{j}")
                for j in range(nchunks)
            ]
            res_tiles = []
            for t in range(nt):
                f_t = ks[t] * C
                if t < res:
                    x = resp.tile([P, F], mybir.dt.float32, name="xr", tag=f"res{t}")
                    res_tiles.append(x)
                    nc.sync.dma_start(out=x[:, :f_t], in_=feat_view(t))
                else:
                    x = inp.tile([P, F], mybir.dt.float32, name="x", tag="x")[:, :f_t]
                    # Pool is idle in pass 1: streamed loads ride SWDGE as a
                    # second descriptor path (SWDGE measured additive, ~346
                    # vs ~328 GB/s on the memcpy probe); a waiting trigger
                    # at Pool's queue head blocks nothing here.
                    nc.gpsimd.dma_start(out=x[:, :f_t], in_=feat_view(t))
                sq = sqp.tile([P, F], mybir.dt.bfloat16, name="sq", tag="sq")
                nc.scalar.activation(sq[:, :f_t], x[:, :f_t], _AFT.Square)
                for j in range((f_t + MM_N - 1) // MM_N):
                    w = min(MM_N, f_t - j * MM_N)
                    nc.tensor.matmul(
                        acc[j][:, :w],
                        lhsT=ones_col[:, :],
                        rhs=sq[:, j * MM_N : j * MM_N + w],
                        start=(t == 0),
                        stop=(t == last_t_for_chunk[j]),
                    )

            # --- combine: [1,64] vector math ------------------------------
            red = small.tile([1, NCHUNK, C], mybir.dt.float32, name="red", tag="red")
            if nchunks < NCHUNK:
                nc.vector.memset(red[:, :, :], 0.0)
            for j in range(nchunks):
                # a chunk may be only partially covered (ragged tail): reduce
                # the written prefix; zero-init handles the rest
                kw = min(MM_N, max(ks) * C - j * MM_N) // C
                nc.vector.tensor_reduce(
                    out=red[:, j, :],
                    in_=acc[j][:, : kw * C].rearrange("p (k c) -> p c k", c=C),
                    axis=mybir.AxisListType.X,
                    op=_ALU.add,
                )
            sumsq = small.tile([1, C], mybir.dt.float32, name="sumsq", tag="sumsq")
            nc.vector.tensor_reduce(
                out=sumsq,
                in_=red[:, :, :].rearrange("p k c -> p c k"),
                axis=mybir.AxisListType.X,
                op=_ALU.add,
            )

            # r2 = 2*sqrt(sumsq) via ACT sqrt + one Newton step (ACT sqrt is
            # low precision; Newton with the accurate DVE reciprocal fixes it)
            r0 = small.tile([1, C], mybir.dt.float32, name="r0", tag="r0")
            nc.scalar.activation(r0, sumsq, _AFT.Sqrt)
            rm = small.tile([1, C], mybir.dt.float32, name="rm", tag="rm")
            nc.vector.tensor_scalar_max(rm, r0, 1e-30)
            rinv = small.tile([1, C], mybir.dt.float32, name="rinv", tag="rinv")
            nc.vector.reciprocal(rinv, rm)
            t1 = small.tile([1, C], mybir.dt.float32, name="t1", tag="t1")
            nc.vector.tensor_mul(t1, sumsq, rinv)
            r2 = small.tile([1, C], mybir.dt.float32, name="r2", tag="r2")
            nc.vector.tensor_add(r2, r0, t1)

            # mean + eps:  me = sum(r2)/128 + EPS   (r2 = 2r -> mean = sum/128)
            msum = small.tile([1, 1], mybir.dt.float32, name="msum", tag="msum")
            nc.vector.tensor_reduce(out=msum, in_=r2, axis=mybir.AxisListType.X, op=_ALU.add)
            eps_t = small.tile([1, 1], mybir.dt.float32, name="eps_t", tag="eps_t")
            nc.vector.memset(eps_t, EPS)
            me = small.tile([1, 1], mybir.dt.float32, name="me", tag="me")
            nc.scalar.activation(me, msum, _AFT.Identity, bias=eps_t[:, :], scale=1.0 / (2 * C))
            minv = small.tile([1, 1], mybir.dt.float32, name="minv", tag="minv")
            nc.vector.reciprocal(minv, me)
            mh = small.tile([1, 1], mybir.dt.float32, name="mh", tag="mh")
            nc.vector.tensor_scalar_mul(mh, minv, 0.5)

            # s = 1 + gamma * (r2 * 0.5 * minv); pack [s | beta] in one row
            g_row = small.tile([1, C], mybir.dt.float32, name="g_row", tag="g_row")
            nc.sync.dma_start(out=g_row, in_=gamma[:])
            t2 = small.tile([1, C], mybir.dt.float32, name="t2", tag="t2")
            nc.vector.tensor_mul(t2, r2, g_row)
            sb_cat = small.tile([1, 2 * C], mybir.dt.float32, name="sb_cat", tag="sb_cat")
            nc.vector.tensor_scalar(
                sb_cat[:, 0:C], t2, scalar1=mh[:, :], scalar2=1.0, op0=_ALU.mult, op1=_ALU.add
            )
            nc.sync.dma_start(out=sb_cat[:, C : 2 * C], in_=beta[:])

            # broadcast [1,128] -> [128,128]: cols 0-63 = s, 64-127 = beta
            bc_ps = psum.tile([P, 2 * C], mybir.dt.float32, name="bc_ps", tag="bc_ps")
            nc.tensor.matmul(bc_ps[:, :], lhsT=ones_row[:, :], rhs=sb_cat[:, :], start=True, stop=True)
            sb_bc = small.tile([P, 2 * C], mybir.dt.float32, name="sb_bc", tag="sb_bc")
            nc.scalar.copy(sb_bc, bc_ps)
            s_bc = sb_bc[:, 0:C]
            b_bc = sb_bc[:, C : 2 * C]

            def bcast_ap(col_slice, kk):
                return bass.AP(
                    tensor=col_slice.tensor,
                    offset=col_slice.offset,
                    ap=[col_slice.ap[0], [0, kk], col_slice.ap[1]],
                )

            def rescale(x, t):
                kk = ks[t]
                x3 = x[:, : kk * C].rearrange("p (k c) -> p k c", c=C)
                nc.vector.tensor_tensor(x3, x3, bcast_ap(s_bc, kk), _ALU.mult)
                # alternate the +beta between Pool and DVE so neither engine
                # becomes the pass-2 critical path
                eng = adder if t % 2 == 0 else nc.vector
                eng.tensor_tensor(x3, x3, bcast_ap(b_bc, kk), _ALU.add)

            # --- pass 2: y = x*s + beta (in place) ------------------------
            # resident tiles first (no loads; fills the combine bubble while
            # streamed loads prefetch), stored from SBUF directly
            for t in range(res):
                x = res_tiles[t]
                rescale(x, t)
                nc.scalar.dma_start(out=out_view(t), in_=x[:, : ks[t] * C])
            for t in range(res, nt):
                f_t = ks[t] * C
                x = inp.tile([P, F], mybir.dt.float32, name="x", tag="x")[:, :f_t]
                nc.sync.dma_start(out=x, in_=feat_view(t))
                rescale(x, t)
                nc.scalar.dma_start(out=out_view(t), in_=x)

    nc.finalize()
    return nc


def kernel(feat: np.ndarray, offset: np.ndarray, gamma: np.ndarray, beta: np.ndarray) -> np.ndarray:
    feat = np.ascontiguousarray(np.asarray(feat, dtype=np.float32))
    offset = np.asarray(offset)
    gamma = np.ascontiguousarray(np.asarray(gamma, dtype=np.float32)).reshape(1, C)
    beta = np.ascontiguousarray(np.asarray(beta, dtype=np.float32)).reshape(1, C)

    n = feat.shape[0]
    b = offset.shape[0]
    assert b <= N_CORES, f"need <= {N_CORES} segments, got {b}"

    ends = offset.astype(np.int64)
    starts = np.concatenate([[0], ends[:-1]])
    seg_rows = (ends - starts).astype(np.int64)

    r_max = int(seg_rows.max()) if b else P
    r_pad = max(P, ((r_max + P - 1) // P) * P)

    key = (r_pad,)
    nc = _program_cache.get(key)
    if nc is None:
        nc = _build_program(r_pad)
        _program_cache[key] = nc

    in_maps = []
    for i in range(N_CORES):
        shard = np.zeros((r_pad, C), dtype=np.float32)
        if i < b and seg_rows[i] > 0:
            shard[: seg_rows[i]] = feat[starts[i] : ends[i]]
        in_maps.append({"feat": shard, "gamma": gamma, "beta": beta})

    results = run_bass_kernel_spmd(nc, in_maps, core_ids=list(range(N_CORES))).results

    out_full = np.empty((n, C), dtype=np.float32)
    for i in range(b):
        if seg_rows[i] > 0:
            out_full[starts[i] : ends[i]] = results[i]["out"][: seg_rows[i]]

    # Rows past offset[-1] (possible with general sorted offsets): the
    # reference's searchsorted yields index b there, which jax clamps to
    # b-1 on gather — those rows are scaled by the last segment's rn but
    # excluded from its sumsq.  Replicate on host.
    tail0 = int(ends[-1]) if b else 0
    if tail0 < n:
        last0, last1 = int(starts[-1]), int(ends[-1])
        sumsq = (feat[last0:last1].astype(np.float64) ** 2).sum(axis=0)
        r = np.sqrt(sumsq)
        rn = (r / (r.mean() + EPS)).astype(np.float32)
        ft = feat[tail0:]
        out_full[tail0:] = ft + gamma * (ft * rn[None, :]) + beta
    return out_full



# revision 2
# speedup vs baseline: 191.1697x; 191.1697x over previous
"""PointGRN (segment_reduce) Trainium2 Bass kernel.

Computation (per segment b, channel c over points feat [N, 64] f32):
    sumsq[b,c]  = sum_{n in seg b} feat[n,c]^2
    r[b,c]      = sqrt(sumsq[b,c])
    rn[b,c]     = r[b,c] / (mean_c r[b,:] + 1e-6)
    out[n,c]    = feat[n,c] * (1 + gamma[c]*rn[b,c]) + beta[c]

Sharding: data-parallel over segments — host reads `offset` and gives each
of the 8 cores one whole segment (padded with zero rows to a 128-row
multiple).  No device-side searchsorted and no collectives needed.

Device kernel (per core), DMA-bound:
    pass 1: stream [128 x k*64] f32 tiles (k=32 plus one ragged tail) on
            the SP + SWDGE rings; ACT squares into bf16; PE ones-matmul
            reduces partitions into 4 PSUM rows; DVE downcasts each tile
            to an fp16 resident copy (122 KB/partition — the whole 32 MB
            shard stays on-chip, so pass 2 re-reads nothing from HBM).
    combine: tiny [1,64] vector math (sqrt + Newton step, mean, scale),
            broadcast scale/beta to [128,128] via a K=1 matmul.
    pass 2: y = fp16_resident * s + beta into f32 staging tiles (mult on
            DVE; +beta alternates Pool/DVE); stores alternate the ACT and
            SP HWDGE rings.  Total HBM traffic 64 MB/core (32 in + 32 out),
            the roofline floor, vs 75 MB for the reload-based variant.
"""

import numpy as np

import concourse.bacc as bacc
import concourse.bass as bass
import concourse.mybir as mybir
import concourse.tile as tile
from concourse.bass_utils import run_bass_kernel_spmd

EPS = 1e-06
N_CORES = 8
P = 128          # SBUF partitions
C = 64           # channels
K = 32           # row-groups per partition per full tile
F = K * C        # full-tile free dim (2048 f32 = 8KB/partition)
TILE_ROWS = P * K  # 4096 rows per full tile
MM_N = 512       # matmul moving free-dim chunk
NCHUNK = F // MM_N

_AFT = mybir.ActivationFunctionType
_ALU = mybir.AluOpType

_program_cache: dict[tuple, bass.Bass] = {}


def _tile_rows(r_pad):
    """Split r_pad rows into full [128 x K] tiles plus one ragged tail tile."""
    pchunks = r_pad // P
    nt_full = pchunks // K
    k_tail = pchunks % K
    ks = [K] * nt_full + ([k_tail] if k_tail else [])
    return ks


def _build_program(
    r_pad: int,
    repeats: int = 1,
    add_eng: str = "gpsimd",
    bufs_x: int = 3,
    bufs_y: int = 3,
) -> bass.Bass:
    """One-core Bass program for a shard of r_pad rows (r_pad % 128 == 0).

    `repeats` re-runs the whole computation body that many times (timing
    only: the wall-clock slope over repeats isolates kernel time from the
    ~80-100ms flat dispatch overhead of this axon environment).
    """
    from contextlib import ExitStack

    ks = _tile_rows(r_pad)
    nt = len(ks)
    nc = bacc.Bacc()

    feat = nc.declare_dram_parameter("feat", [r_pad, C], mybir.dt.float32, isOutput=False)
    gamma = nc.declare_dram_parameter("gamma", [1, C], mybir.dt.float32, isOutput=False)
    beta = nc.declare_dram_parameter("beta", [1, C], mybir.dt.float32, isOutput=False)
    out = nc.declare_dram_parameter("out", [r_pad, C], mybir.dt.float32, isOutput=True)

    row0 = [0] * nt
    for t in range(1, nt):
        row0[t] = row0[t - 1] + P * ks[t - 1]

    def feat_view(t):
        r0 = row0[t]
        return feat[r0 : r0 + P * ks[t], :].rearrange("(p k) c -> p (k c)", k=ks[t])

    def out_view(t):
        r0 = row0[t]
        return out[r0 : r0 + P * ks[t], :].rearrange("(p k) c -> p (k c)", k=ks[t])

    with tile.TileContext(nc) as tc, ExitStack() as ctx:
        const = ctx.enter_context(tc.tile_pool(name="const", bufs=1))
        inp = ctx.enter_context(tc.tile_pool(name="inp", bufs=bufs_x))
        outp = ctx.enter_context(tc.tile_pool(name="outp", bufs=bufs_y))
        xhp = ctx.enter_context(tc.tile_pool(name="xhp", bufs=1))
        sqp = ctx.enter_context(tc.tile_pool(name="sqp", bufs=2))
        psum = ctx.enter_context(tc.tile_pool(name="psum", bufs=1, space="PSUM"))
        small = ctx.enter_context(tc.tile_pool(name="small", bufs=1))
        adder = getattr(nc, add_eng)

        ones_col = const.tile([P, 1], mybir.dt.bfloat16, name="ones_col", tag="ones_col")
        nc.vector.memset(ones_col, 1.0)
        ones_row = const.tile([1, P], mybir.dt.float32, name="ones_row", tag="ones_row")
        nc.vector.memset(ones_row, 1.0)

        # chunks actually written, and the last tile writing each (stop flag)
        nchunks = (max(ks) * C + MM_N - 1) // MM_N
        last_t_for_chunk = [0] * nchunks
        for t in range(nt):
            for j in range((ks[t] * C + MM_N - 1) // MM_N):
                last_t_for_chunk[j] = t

        for _rep in range(repeats):
            # --- pass 1: sum of squares + fp16 residency ------------------
            acc = [
                psum.tile([1, MM_N], mybir.dt.float32, name=f"acc{j}", tag=f"acc{j}")
                for j in range(nchunks)
            ]
            xh_tiles = []
            for t in range(nt):
                f_t = ks[t] * C
                x = inp.tile([P, F], mybir.dt.float32, name="x", tag="x")[:, :f_t]
                eng = nc.sync if t % 2 == 0 else nc.gpsimd
                eng.dma_start(out=x, in_=feat_view(t))
                sq = sqp.tile([P, F], mybir.dt.bfloat16, name="sq", tag="sq")
                nc.scalar.activation(sq[:, :f_t], x, _AFT.Square)
                xh = xhp.tile([P, f_t], mybir.dt.float16, name="xh", tag=f"xh{t}")
                nc.vector.tensor_copy(xh, x)
                xh_tiles.append(xh)
                for j in range((f_t + MM_N - 1) // MM_N):
                    w = min(MM_N, f_t - j * MM_N)
                    nc.tensor.matmul(
                        acc[j][:, :w],
                        lhsT=ones_col[:, :],
                        rhs=sq[:, j * MM_N : j * MM_N + w],
                        start=(t == 0),
                        stop=(t == last_t_for_chunk[j]),
                    )

            # --- combine: [1,64] vector math ------------------------------
            red = small.tile([1, NCHUNK, C], mybir.dt.float32, name="red", tag="red")
            if nchunks < NCHUNK:
                nc.vector.memset(red[:, :, :], 0.0)
            for j in range(nchunks):
                # a chunk may be only partially covered (ragged tail): reduce
                # the written prefix; zero-init handles the rest
                kw = min(MM_N, max(ks) * C - j * MM_N) // C
                nc.vector.tensor_reduce(
                    out=red[:, j, :],
                    in_=acc[j][:, : kw * C].rearrange("p (k c) -> p c k", c=C),
                    axis=mybir.AxisListType.X,
                    op=_ALU.add,
                )
            sumsq = small.tile([1, C], mybir.dt.float32, name="sumsq", tag="sumsq")
            nc.vector.tensor_reduce(
                out=sumsq,
                in_=red[:, :, :].rearrange("p k c -> p c k"),
                axis=mybir.AxisListType.X,
                op=_ALU.add,
            )

            # r2 = 2*sqrt(sumsq) via ACT sqrt + one Newton step (ACT sqrt is
            # low precision; Newton with the accurate DVE reciprocal fixes it)
            r0 = small.tile([1, C], mybir.dt.float32, name="r0", tag="r0")
            nc.scalar.activation(r0, sumsq, _AFT.Sqrt)
            rm = small.tile([1, C], mybir.dt.float32, name="rm", tag="rm")
            nc.vector.tensor_scalar_max(rm, r0, 1e-30)
            rinv = small.tile([1, C], mybir.dt.float32, name="rinv", tag="rinv")
            nc.vector.reciprocal(rinv, rm)
            t1 = small.tile([1, C], mybir.dt.float32, name="t1", tag="t1")
            nc.vector.tensor_mul(t1, sumsq, rinv)
            r2 = small.tile([1, C], mybir.dt.float32, name="r2", tag="r2")
            nc.vector.tensor_add(r2, r0, t1)

            # mean + eps:  me = sum(r2)/128 + EPS   (r2 = 2r -> mean = sum/128)
            msum = small.tile([1, 1], mybir.dt.float32, name="msum", tag="msum")
            nc.vector.tensor_reduce(out=msum, in_=r2, axis=mybir.AxisListType.X, op=_ALU.add)
            eps_t = small.tile([1, 1], mybir.dt.float32, name="eps_t", tag="eps_t")
            nc.vector.memset(eps_t, EPS)
            me = small.tile([1, 1], mybir.dt.float32, name="me", tag="me")
            nc.scalar.activation(me, msum, _AFT.Identity, bias=eps_t[:, :], scale=1.0 / (2 * C))
            minv = small.tile([1, 1], mybir.dt.float32, name="minv", tag="minv")
            nc.vector.reciprocal(minv, me)
            mh = small.tile([1, 1], mybir.dt.float32, name="mh", tag="mh")
            nc.vector.tensor_scalar_mul(mh, minv, 0.5)

            # s = 1 + gamma * (r2 * 0.5 * minv); pack [s | beta] in one row
            g_row = small.tile([1, C], mybir.dt.float32, name="g_row", tag="g_row")
            nc.sync.dma_start(out=g_row, in_=gamma[:])
            t2 = small.tile([1, C], mybir.dt.float32, name="t2", tag="t2")
            nc.vector.tensor_mul(t2, r2, g_row)
            sb_cat = small.tile([1, 2 * C], mybir.dt.float32, name="sb_cat", tag="sb_cat")
            nc.vector.tensor_scalar(
                sb_cat[:, 0:C], t2, scalar1=mh[:, :], scalar2=1.0, op0=_ALU.mult, op1=_ALU.add
            )
            nc.sync.dma_start(out=sb_cat[:, C : 2 * C], in_=beta[:])

            # broadcast [1,128] -> [128,128]: cols 0-63 = s, 64-127 = beta
            bc_ps = psum.tile([P, 2 * C], mybir.dt.float32, name="bc_ps", tag="bc_ps")
            nc.tensor.matmul(bc_ps[:, :], lhsT=ones_row[:, :], rhs=sb_cat[:, :], start=True, stop=True)
            sb_bc = small.tile([P, 2 * C], mybir.dt.float32, name="sb_bc", tag="sb_bc")
            nc.scalar.copy(sb_bc, bc_ps)
            s_bc = sb_bc[:, 0:C]
            b_bc = sb_bc[:, C : 2 * C]

            def bcast_ap(col_slice, kk):
                return bass.AP(
                    tensor=col_slice.tensor,
                    offset=col_slice.offset,
                    ap=[col_slice.ap[0], [0, kk], col_slice.ap[1]],
                )

            # --- pass 2: y = xh*s + beta into staging, store --------------
            for t in range(nt):
                f_t = ks[t] * C
                kk = ks[t]
                y = outp.tile([P, F], mybir.dt.float32, name="y", tag="y")[:, :f_t]
                y3 = y.rearrange("p (k c) -> p k c", c=C)
                xh3 = xh_tiles[t].rearrange("p (k c) -> p k c", c=C)
                nc.vector.tensor_tensor(y3, xh3, bcast_ap(s_bc, kk), _ALU.mult)
                # alternate the +beta between Pool and DVE so neither engine
                # becomes the pass-2 critical path
                eng = adder if t % 2 == 0 else nc.vector
                eng.tensor_tensor(y3, y3, bcast_ap(b_bc, kk), _ALU.add)
                seng = nc.scalar if t % 2 == 0 else nc.sync
                seng.dma_start(out=out_view(t), in_=y)

    nc.finalize()
    return nc


def kernel(feat: np.ndarray, offset: np.ndarray, gamma: np.ndarray, beta: np.ndarray) -> np.ndarray:
    feat = np.ascontiguousarray(np.asarray(feat, dtype=np.float32))
    offset = np.asarray(offset)
    gamma = np.ascontiguousarray(np.asarray(gamma, dtype=np.float32)).reshape(1, C)
    beta = np.ascontiguousarray(np.asarray(beta, dtype=np.float32)).reshape(1, C)

    n = feat.shape[0]
    b = offset.shape[0]
    assert b <= N_CORES, f"need <= {N_CORES} segments, got {b}"

    ends = offset.astype(np.int64)
    starts = np.concatenate([[0], ends[:-1]])
    seg_rows = (ends - starts).astype(np.int64)

    r_max = int(seg_rows.max()) if b else P
    r_pad = max(P, ((r_max + P - 1) // P) * P)

    key = (r_pad,)
    nc = _program_cache.get(key)
    if nc is None:
        nc = _build_program(r_pad)
        _program_cache[key] = nc

    in_maps = []
    for i in range(N_CORES):
        shard = np.zeros((r_pad, C), dtype=np.float32)
        if i < b and seg_rows[i] > 0:
            shard[: seg_rows[i]] = feat[starts[i] : ends[i]]
        in_maps.append({"feat": shard, "gamma": gamma, "beta": beta})

    results = run_bass_kernel_spmd(nc, in_maps, core_ids=list(range(N_CORES))).results

    out_full = np.empty((n, C), dtype=np.float32)
    for i in range(b):
        if seg_rows[i] > 0:
            out_full[starts[i] : ends[i]] = results[i]["out"][: seg_rows[i]]

    # Rows past offset[-1] (possible with general sorted offsets): the
    # reference's searchsorted yields index b there, which jax clamps to
    # b-1 on gather — those rows are scaled by the last segment's rn but
    # excluded from its sumsq.  Replicate on host.
    tail0 = int(ends[-1]) if b else 0
    if tail0 < n:
        last0, last1 = int(starts[-1]), int(ends[-1])
        sumsq = (feat[last0:last1].astype(np.float64) ** 2).sum(axis=0)
        r = np.sqrt(sumsq)
        rn = (r / (r.mean() + EPS)).astype(np.float32)
        ft = feat[tail0:]
        out_full[tail0:] = ft + gamma * (ft * rn[None, :]) + beta
    return out_full


# revision 13
# speedup vs baseline: 270.4409x; 1.4147x over previous
"""PointGRN (segment_reduce) Trainium2 Bass kernel.

Computation (per segment b, channel c over points feat [N, 64] f32):
    sumsq[b,c]  = sum_{n in seg b} feat[n,c]^2
    r[b,c]      = sqrt(sumsq[b,c])
    rn[b,c]     = r[b,c] / (mean_c r[b,:] + 1e-6)
    out[n,c]    = feat[n,c] * (1 + gamma[c]*rn[b,c]) + beta[c]

Sharding: data-parallel over segments — host reads `offset` and gives each
of the 8 cores one whole segment (padded with zero rows to a 128-row
multiple).  No device-side searchsorted and no collectives needed.

Device kernel (per core), DMA-bound:
    pass 1: SWDGE cast-loads stream each [128 x k*64] tile f32->fp16
            straight into its own resident SBUF slot (122 KB/partition
            total — the whole 32 MB shard stays on-chip, no input pool,
            no buffer-recycle stalls); ACT squares the fp16 copy into
            bf16; PE ones-matmul reduces partitions into 4 PSUM rows.
    combine: tiny [1,64] vector math (sqrt + Newton step, mean, scale),
            broadcast scale/beta to [128,128] via a K=1 matmul.
    pass 2: y = fp16_resident * s + beta into f32 staging tiles (mult on
            DVE; +beta mostly on Pool, a few on DVE to balance); stores
            alternate the ACT and SP HWDGE rings.  Total HBM traffic
            64 MB/core (32 in + 32 out), the roofline floor, vs 75 MB
            for the reload-based variant.
"""

import numpy as np

import concourse.bacc as bacc
import concourse.bass as bass
import concourse.mybir as mybir
import concourse.tile as tile
from concourse.bass_utils import run_bass_kernel_spmd

EPS = 1e-06
N_CORES = 8
P = 128          # SBUF partitions
C = 64           # channels
K = 32           # row-groups per partition per full tile
F = K * C        # full-tile free dim (2048 f32 = 8KB/partition)
TILE_ROWS = P * K  # 4096 rows per full tile
MM_N = 512       # matmul moving free-dim chunk
NCHUNK = F // MM_N

_AFT = mybir.ActivationFunctionType
_ALU = mybir.AluOpType

_program_cache: dict[tuple, bass.Bass] = {}


def _tile_rows(r_pad):
    """Split r_pad rows into full [128 x K] tiles plus one ragged tail tile."""
    pchunks = r_pad // P
    nt_full = pchunks // K
    k_tail = pchunks % K
    ks = [K] * nt_full + ([k_tail] if k_tail else [])
    return ks


def _build_program(
    r_pad: int,
    repeats: int = 1,
    add_eng: str = "gpsimd",
    dve_adds: int = 14,
    bufs_y: int = 5,
    cast_store: bool = False,
) -> bass.Bass:
    """One-core Bass program for a shard of r_pad rows (r_pad % 128 == 0).

    `repeats` re-runs the whole computation body that many times (timing
    only: the wall-clock slope over repeats isolates kernel time from the
    ~80-100ms flat dispatch overhead of this axon environment).
    """
    from contextlib import ExitStack

    ks = _tile_rows(r_pad)
    nt = len(ks)
    nc = bacc.Bacc()

    feat = nc.declare_dram_parameter("feat", [r_pad, C], mybir.dt.float32, isOutput=False)
    gamma = nc.declare_dram_parameter("gamma", [1, C], mybir.dt.float32, isOutput=False)
    beta = nc.declare_dram_parameter("beta", [1, C], mybir.dt.float32, isOutput=False)
    out = nc.declare_dram_parameter("out", [r_pad, C], mybir.dt.float32, isOutput=True)

    row0 = [0] * nt
    for t in range(1, nt):
        row0[t] = row0[t - 1] + P * ks[t - 1]

    def feat_view(t):
        r0 = row0[t]
        return feat[r0 : r0 + P * ks[t], :].rearrange("(p k) c -> p (k c)", k=ks[t])

    def out_view(t):
        r0 = row0[t]
        return out[r0 : r0 + P * ks[t], :].rearrange("(p k) c -> p (k c)", k=ks[t])

    with tile.TileContext(nc) as tc, ExitStack() as ctx:
        const = ctx.enter_context(tc.tile_pool(name="const", bufs=1))
        outp = ctx.enter_context(tc.tile_pool(name="outp", bufs=bufs_y))
        xhp = ctx.enter_context(tc.tile_pool(name="xhp", bufs=1))
        sqp = ctx.enter_context(tc.tile_pool(name="sqp", bufs=3))
        psum = ctx.enter_context(tc.tile_pool(name="psum", bufs=1, space="PSUM"))
        small = ctx.enter_context(tc.tile_pool(name="small", bufs=1))
        adder = getattr(nc, add_eng)

        ones_col = const.tile([P, 1], mybir.dt.bfloat16, name="ones_col", tag="ones_col")
        nc.vector.memset(ones_col, 1.0)
        ones_row = const.tile([1, P], mybir.dt.float32, name="ones_row", tag="ones_row")
        nc.vector.memset(ones_row, 1.0)

        # chunks actually written, and the last tile writing each (stop flag)
        nchunks = (max(ks) * C + MM_N - 1) // MM_N
        last_t_for_chunk = [0] * nchunks
        for t in range(nt):
            for j in range((ks[t] * C + MM_N - 1) // MM_N):
                last_t_for_chunk[j] = t

        for _rep in range(repeats):
            # --- pass 1: sum of squares + fp16 residency ------------------
            acc = [
                psum.tile([1, MM_N], mybir.dt.float32, name=f"acc{j}", tag=f"acc{j}")
                for j in range(nchunks)
            ]
            xh_tiles = []
            for t in range(nt):
                f_t = ks[t] * C
                # SWDGE cast-load: f32 HBM -> fp16 resident slot in one DMA
                xh = xhp.tile([P, f_t], mybir.dt.float16, name="xh", tag=f"xh{t}")
                nc.gpsimd.dma_start(out=xh, in_=feat_view(t))
                xh_tiles.append(xh)
                sq = sqp.tile([P, F], mybir.dt.bfloat16, name="sq", tag="sq")
                nc.vector.tensor_mul(sq[:, :f_t], xh, xh)
                for j in range((f_t + MM_N - 1) // MM_N):
                    w = min(MM_N, f_t - j * MM_N)
                    nc.tensor.matmul(
                        acc[j][:, :w],
                        lhsT=ones_col[:, :],
                        rhs=sq[:, j * MM_N : j * MM_N + w],
                        start=(t == 0),
                        stop=(t == last_t_for_chunk[j]),
                    )

            # --- combine: [1,64] vector math ------------------------------
            red = small.tile([1, NCHUNK, C], mybir.dt.float32, name="red", tag="red")
            if nchunks < NCHUNK:
                nc.vector.memset(red[:, :, :], 0.0)
            for j in range(nchunks):
                # a chunk may be only partially covered (ragged tail): reduce
                # the written prefix; zero-init handles the rest
                kw = min(MM_N, max(ks) * C - j * MM_N) // C
                nc.vector.tensor_reduce(
                    out=red[:, j, :],
                    in_=acc[j][:, : kw * C].rearrange("p (k c) -> p c k", c=C),
                    axis=mybir.AxisListType.X,
                    op=_ALU.add,
                )
            sumsq = small.tile([1, C], mybir.dt.float32, name="sumsq", tag="sumsq")
            nc.vector.tensor_reduce(
                out=sumsq,
                in_=red[:, :, :].rearrange("p k c -> p c k"),
                axis=mybir.AxisListType.X,
                op=_ALU.add,
            )

            # r2 = 2*sqrt(sumsq) via ACT sqrt + one Newton step (ACT sqrt is
            # low precision; Newton with the accurate DVE reciprocal fixes it)
            r0 = small.tile([1, C], mybir.dt.float32, name="r0", tag="r0")
            nc.scalar.activation(r0, sumsq, _AFT.Sqrt)
            rm = small.tile([1, C], mybir.dt.float32, name="rm", tag="rm")
            nc.vector.tensor_scalar_max(rm, r0, 1e-30)
            rinv = small.tile([1, C], mybir.dt.float32, name="rinv", tag="rinv")
            nc.vector.reciprocal(rinv, rm)
            t1 = small.tile([1, C], mybir.dt.float32, name="t1", tag="t1")
            nc.vector.tensor_mul(t1, sumsq, rinv)
            r2 = small.tile([1, C], mybir.dt.float32, name="r2", tag="r2")
            nc.vector.tensor_add(r2, r0, t1)

            # mean + eps:  me = sum(r2)/128 + EPS   (r2 = 2r -> mean = sum/128)
            msum = small.tile([1, 1], mybir.dt.float32, name="msum", tag="msum")
            nc.vector.tensor_reduce(out=msum, in_=r2, axis=mybir.AxisListType.X, op=_ALU.add)
            eps_t = small.tile([1, 1], mybir.dt.float32, name="eps_t", tag="eps_t")
            nc.vector.memset(eps_t, EPS)
            me = small.tile([1, 1], mybir.dt.float32, name="me", tag="me")
            nc.scalar.activation(me, msum, _AFT.Identity, bias=eps_t[:, :], scale=1.0 / (2 * C))
            minv = small.tile([1, 1], mybir.dt.float32, name="minv", tag="minv")
            nc.vector.reciprocal(minv, me)
            mh = small.tile([1, 1], mybir.dt.float32, name="mh", tag="mh")
            nc.vector.tensor_scalar_mul(mh, minv, 0.5)

            # s = 1 + gamma * (r2 * 0.5 * minv); pack [s | beta] in one row
            g_row = small.tile([1, C], mybir.dt.float32, name="g_row", tag="g_row")
            nc.sync.dma_start(out=g_row, in_=gamma[:])
            t2 = small.tile([1, C], mybir.dt.float32, name="t2", tag="t2")
            nc.vector.tensor_mul(t2, r2, g_row)
            sb_cat = small.tile([1, 2 * C], mybir.dt.float32, name="sb_cat", tag="sb_cat")
            nc.vector.tensor_scalar(
                sb_cat[:, 0:C], t2, scalar1=mh[:, :], scalar2=1.0, op0=_ALU.mult, op1=_ALU.add
            )
            nc.sync.dma_start(out=sb_cat[:, C : 2 * C], in_=beta[:])

            # broadcast [1,128] -> [128,128]: cols 0-63 = s, 64-127 = beta
            bc_ps = psum.tile([P, 2 * C], mybir.dt.float32, name="bc_ps", tag="bc_ps")
            nc.tensor.matmul(bc_ps[:, :], lhsT=ones_row[:, :], rhs=sb_cat[:, :], start=True, stop=True)
            sb_bc = small.tile([P, 2 * C], mybir.dt.float32, name="sb_bc", tag="sb_bc")
            nc.scalar.copy(sb_bc, bc_ps)
            s_bc = sb_bc[:, 0:C]
            b_bc = sb_bc[:, C : 2 * C]

            def bcast_ap(col_slice, kk):
                return bass.AP(
                    tensor=col_slice.tensor,
                    offset=col_slice.offset,
                    ap=[col_slice.ap[0], [0, kk], col_slice.ap[1]],
                )

            # materialize contiguous [128, F] repeats of s (fp16) and beta
            # (f32): every pass-2 tensor op then runs on flat 2D operands
            # instead of stride-0 broadcast APs (which DVE executes ~1.5x
            # slower and Pool ~1.4x slower per element)
            s_rep = small.tile([P, K, C], mybir.dt.float16, name="s_rep", tag="s_rep")
            nc.vector.tensor_copy(s_rep, bcast_ap(s_bc, K))
            b_dt = mybir.dt.float16 if cast_store else mybir.dt.float32
            b_rep = small.tile([P, K, C], b_dt, name="b_rep", tag="b_rep")
            nc.scalar.activation(b_rep, bcast_ap(b_bc, K), _AFT.Identity)
            s_rep2 = s_rep.rearrange("p k c -> p (k c)")
            b_rep2 = b_rep.rearrange("p k c -> p (k c)")

            # --- pass 2: y = xh*s + beta, store ---------------------------
            # in-place fp16 mult on the (now dead) resident tile.  Then
            # either (cast_store) an in-place fp16 +beta and a SWDGE
            # cast-store straight from the resident tile, or a +beta that
            # upconverts into an f32 staging tile stored via HWDGE.  Adds
            # split DVE/Pool (Pool is ~2x slower per op but otherwise idle).
            for t in range(nt):
                f_t = ks[t] * C
                xh = xh_tiles[t]
                nc.vector.tensor_mul(xh, xh, s_rep2[:, :f_t])
                eng = nc.vector if (t * dve_adds) % nt < dve_adds else adder
                if cast_store:
                    eng.tensor_tensor(xh, xh, b_rep2[:, :f_t], _ALU.add)
                    nc.gpsimd.dma_start(out=out_view(t), in_=xh)
                else:
                    y = outp.tile([P, F], mybir.dt.float32, name="y", tag="y")[:, :f_t]
                    eng.tensor_tensor(y, xh, b_rep2[:, :f_t], _ALU.add)
                    seng = nc.scalar if t % 2 == 0 else nc.sync
                    seng.dma_start(out=out_view(t), in_=y)

    nc.finalize()
    return nc


def kernel(feat: np.ndarray, offset: np.ndarray, gamma: np.ndarray, beta: np.ndarray) -> np.ndarray:
    feat = np.ascontiguousarray(np.asarray(feat, dtype=np.float32))
    offset = np.asarray(offset)
    gamma = np.ascontiguousarray(np.asarray(gamma, dtype=np.float32)).reshape(1, C)
    beta = np.ascontiguousarray(np.asarray(beta, dtype=np.float32)).reshape(1, C)

    n = feat.shape[0]
    b = offset.shape[0]
    assert b <= N_CORES, f"need <= {N_CORES} segments, got {b}"

    ends = offset.astype(np.int64)
    starts = np.concatenate([[0], ends[:-1]])
    seg_rows = (ends - starts).astype(np.int64)

    r_max = int(seg_rows.max()) if b else P
    r_pad = max(P, ((r_max + P - 1) // P) * P)

    key = (r_pad,)
    nc = _program_cache.get(key)
    if nc is None:
        nc = _build_program(r_pad)
        _program_cache[key] = nc

    in_maps = []
    for i in range(N_CORES):
        shard = np.zeros((r_pad, C), dtype=np.float32)
        if i < b and seg_rows[i] > 0:
            shard[: seg_rows[i]] = feat[starts[i] : ends[i]]
        in_maps.append({"feat": shard, "gamma": gamma, "beta": beta})

    results = run_bass_kernel_spmd(nc, in_maps, core_ids=list(range(N_CORES))).results

    out_full = np.empty((n, C), dtype=np.float32)
    for i in range(b):
        if seg_rows[i] > 0:
            out_full[starts[i] : ends[i]] = results[i]["out"][: seg_rows[i]]

    # Rows past offset[-1] (possible with general sorted offsets): the
    # reference's searchsorted yields index b there, which jax clamps to
    # b-1 on gather — those rows are scaled by the last segment's rn but
    # excluded from its sumsq.  Replicate on host.
    tail0 = int(ends[-1]) if b else 0
    if tail0 < n:
        last0, last1 = int(starts[-1]), int(ends[-1])
        sumsq = (feat[last0:last1].astype(np.float64) ** 2).sum(axis=0)
        r = np.sqrt(sumsq)
        rn = (r / (r.mean() + EPS)).astype(np.float32)
        ft = feat[tail0:]
        out_full[tail0:] = ft + gamma * (ft * rn[None, :]) + beta
    return out_full


# revision 23
# speedup vs baseline: 286.0966x; 1.0579x over previous
"""PointGRN (segment_reduce) Trainium2 Bass kernel.

Computation (per segment b, channel c over points feat [N, 64] f32):
    sumsq[b,c]  = sum_{n in seg b} feat[n,c]^2
    r[b,c]      = sqrt(sumsq[b,c])
    rn[b,c]     = r[b,c] / (mean_c r[b,:] + 1e-6)
    out[n,c]    = feat[n,c] * (1 + gamma[c]*rn[b,c]) + beta[c]

Sharding: data-parallel over segments — host reads `offset` and gives each
of the 8 cores one whole segment (padded with zero rows to a 128-row
multiple).  No device-side searchsorted and no collectives needed.

Device kernel (per core), DMA-bound:
    pass 1: SWDGE cast-loads stream each [128 x k*64] tile f32->fp16
            straight into its own resident SBUF slot (122 KB/partition
            total — the whole 32 MB shard stays on-chip, no input pool,
            no buffer-recycle stalls); ACT squares the fp16 copy into
            bf16; PE ones-matmul reduces partitions into 4 PSUM rows.
    combine: tiny [1,64] vector math (sqrt + Newton step, mean, scale),
            broadcast scale/beta to [128,128] via a K=1 matmul.
    pass 2: y = fp16_resident * s + beta into f32 staging tiles (mult on
            DVE; +beta mostly on Pool, a few on DVE to balance); stores
            alternate the ACT and SP HWDGE rings.  Total HBM traffic
            64 MB/core (32 in + 32 out), the roofline floor, vs 75 MB
            for the reload-based variant.
"""

import numpy as np

import concourse.bacc as bacc
import concourse.bass as bass
import concourse.mybir as mybir
import concourse.tile as tile
from concourse.bass_utils import run_bass_kernel_spmd

EPS = 1e-06
N_CORES = 8
P = 128          # SBUF partitions
C = 64           # channels
K = 32           # row-groups per partition per full tile
F = K * C        # full-tile free dim (2048 f32 = 8KB/partition)
TILE_ROWS = P * K  # 4096 rows per full tile
MM_N = 512       # matmul moving free-dim chunk
NCHUNK = F // MM_N

_AFT = mybir.ActivationFunctionType
_ALU = mybir.AluOpType

_program_cache: dict[tuple, bass.Bass] = {}

# device-kernel variant used by kernel(); ab.py sweeps these
CONFIG = dict(cast_store=True, dve_adds=18, act_sqs=25, host_fp16=True)


def _tile_rows(r_pad):
    """Split r_pad rows into full [128 x K] tiles plus one ragged tail tile."""
    pchunks = r_pad // P
    nt_full = pchunks // K
    k_tail = pchunks % K
    ks = [K] * nt_full + ([k_tail] if k_tail else [])
    return ks


def _build_program(
    r_pad: int,
    repeats: int = 1,
    add_eng: str = "gpsimd",
    dve_adds: int = 14,
    bufs_y: int = 5,
    cast_store: bool = False,
    act_sqs: int = 0,
    host_fp16: bool = False,
) -> bass.Bass:
    """One-core Bass program for a shard of r_pad rows (r_pad % 128 == 0).

    `repeats` re-runs the whole computation body that many times (timing
    only: the wall-clock slope over repeats isolates kernel time from the
    ~80-100ms flat dispatch overhead of this axon environment).
    """
    from contextlib import ExitStack

    ks = _tile_rows(r_pad)
    nt = len(ks)
    nc = bacc.Bacc()

    feat_dt = mybir.dt.float16 if host_fp16 else mybir.dt.float32
    feat = nc.declare_dram_parameter("feat", [r_pad, C], feat_dt, isOutput=False)
    gamma = nc.declare_dram_parameter("gamma", [1, C], mybir.dt.float32, isOutput=False)
    beta = nc.declare_dram_parameter("beta", [1, C], mybir.dt.float32, isOutput=False)
    out = nc.declare_dram_parameter("out", [r_pad, C], mybir.dt.float32, isOutput=True)

    row0 = [0] * nt
    for t in range(1, nt):
        row0[t] = row0[t - 1] + P * ks[t - 1]

    def feat_view(t):
        r0 = row0[t]
        return feat[r0 : r0 + P * ks[t], :].rearrange("(p k) c -> p (k c)", k=ks[t])

    def out_view(t):
        r0 = row0[t]
        return out[r0 : r0 + P * ks[t], :].rearrange("(p k) c -> p (k c)", k=ks[t])

    with tile.TileContext(nc) as tc, ExitStack() as ctx:
        const = ctx.enter_context(tc.tile_pool(name="const", bufs=1))
        outp = ctx.enter_context(tc.tile_pool(name="outp", bufs=bufs_y))
        xhp = ctx.enter_context(tc.tile_pool(name="xhp", bufs=1))
        sqp = ctx.enter_context(tc.tile_pool(name="sqp", bufs=3))
        psum = ctx.enter_context(tc.tile_pool(name="psum", bufs=1, space="PSUM"))
        small = ctx.enter_context(tc.tile_pool(name="small", bufs=1))
        adder = getattr(nc, add_eng)

        ones_col = const.tile([P, 1], mybir.dt.bfloat16, name="ones_col", tag="ones_col")
        nc.vector.memset(ones_col, 1.0)
        ones_row = const.tile([1, P], mybir.dt.float32, name="ones_row", tag="ones_row")
        nc.vector.memset(ones_row, 1.0)

        # chunks actually written, and the last tile writing each (stop flag)
        nchunks = (max(ks) * C + MM_N - 1) // MM_N
        last_t_for_chunk = [0] * nchunks
        for t in range(nt):
            for j in range((ks[t] * C + MM_N - 1) // MM_N):
                last_t_for_chunk[j] = t

        def bcast_ap(col_slice, kk):
            return bass.AP(
                tensor=col_slice.tensor,
                offset=col_slice.offset,
                ap=[col_slice.ap[0], [0, kk], col_slice.ap[1]],
            )

        for _rep in range(repeats):
            # beta's [128, F] broadcast only needs the beta DMA — build it
            # during pass 1 (PE + one ACT op, both idle early) so the
            # combine bubble between the passes carries the s-chain only
            b_row = small.tile([1, C], mybir.dt.float32, name="b_row", tag="b_row")
            nc.sync.dma_start(out=b_row, in_=beta[:])
            b_ps = psum.tile([P, C], mybir.dt.float32, name="b_ps", tag="b_ps")
            nc.tensor.matmul(b_ps[:, :], lhsT=ones_row[:, :], rhs=b_row[:, :], start=True, stop=True)
            b_dt = mybir.dt.float16 if cast_store else mybir.dt.float32
            b_rep = small.tile([P, K, C], b_dt, name="b_rep", tag="b_rep")
            nc.scalar.activation(b_rep, bcast_ap(b_ps, K), _AFT.Identity)
            b_rep2 = b_rep.rearrange("p k c -> p (k c)")

            # --- pass 1: sum of squares + fp16 residency ------------------
            acc = [
                psum.tile([1, MM_N], mybir.dt.float32, name=f"acc{j}", tag=f"acc{j}")
                for j in range(nchunks)
            ]
            xh_tiles = []
            for t in range(nt):
                f_t = ks[t] * C
                # SWDGE cast-load: f32 HBM -> fp16 resident slot in one DMA
                xh = xhp.tile([P, f_t], mybir.dt.float16, name="xh", tag=f"xh{t}")
                if host_fp16:
                    # dtypes match: straight load, HWDGE-eligible
                    leng = nc.sync if t % 2 == 0 else nc.scalar
                    leng.dma_start(out=xh, in_=feat_view(t))
                else:
                    # SWDGE cast-load: f32 HBM -> fp16 SBUF in one DMA
                    nc.gpsimd.dma_start(out=xh, in_=feat_view(t))
                xh_tiles.append(xh)
                sq = sqp.tile([P, F], mybir.dt.bfloat16, name="sq", tag="sq")
                # squares: ACT for act_sqs tiles (otherwise idle), DVE rest
                if (t * act_sqs) % nt < act_sqs:
                    nc.scalar.activation(sq[:, :f_t], xh, _AFT.Square)
                else:
                    nc.vector.tensor_mul(sq[:, :f_t], xh, xh)
                for j in range((f_t + MM_N - 1) // MM_N):
                    w = min(MM_N, f_t - j * MM_N)
                    nc.tensor.matmul(
                        acc[j][:, :w],
                        lhsT=ones_col[:, :],
                        rhs=sq[:, j * MM_N : j * MM_N + w],
                        start=(t == 0),
                        stop=(t == last_t_for_chunk[j]),
                    )

            # --- combine: [1,64] vector math ------------------------------
            red = small.tile([1, NCHUNK, C], mybir.dt.float32, name="red", tag="red")
            if nchunks < NCHUNK:
                nc.vector.memset(red[:, :, :], 0.0)
            for j in range(nchunks):
                # a chunk may be only partially covered (ragged tail): reduce
                # the written prefix; zero-init handles the rest
                kw = min(MM_N, max(ks) * C - j * MM_N) // C
                nc.vector.tensor_reduce(
                    out=red[:, j, :],
                    in_=acc[j][:, : kw * C].rearrange("p (k c) -> p c k", c=C),
                    axis=mybir.AxisListType.X,
                    op=_ALU.add,
                )
            sumsq = small.tile([1, C], mybir.dt.float32, name="sumsq", tag="sumsq")
            nc.vector.tensor_reduce(
                out=sumsq,
                in_=red[:, :, :].rearrange("p k c -> p c k"),
                axis=mybir.AxisListType.X,
                op=_ALU.add,
            )

            # r2 = 2*sqrt(sumsq) via ACT sqrt + one Newton step (ACT sqrt is
            # low precision; Newton with the accurate DVE reciprocal fixes it)
            r0 = small.tile([1, C], mybir.dt.float32, name="r0", tag="r0")
            nc.scalar.activation(r0, sumsq, _AFT.Sqrt)
            rm = small.tile([1, C], mybir.dt.float32, name="rm", tag="rm")
            nc.vector.tensor_scalar_max(rm, r0, 1e-30)
            rinv = small.tile([1, C], mybir.dt.float32, name="rinv", tag="rinv")
            nc.vector.reciprocal(rinv, rm)
            t1 = small.tile([1, C], mybir.dt.float32, name="t1", tag="t1")
            nc.vector.tensor_mul(t1, sumsq, rinv)
            r2 = small.tile([1, C], mybir.dt.float32, name="r2", tag="r2")
            nc.vector.tensor_add(r2, r0, t1)

            # mean + eps:  me = sum(r2)/128 + EPS   (r2 = 2r -> mean = sum/128)
            msum = small.tile([1, 1], mybir.dt.float32, name="msum", tag="msum")
            nc.vector.tensor_reduce(out=msum, in_=r2, axis=mybir.AxisListType.X, op=_ALU.add)
            eps_t = small.tile([1, 1], mybir.dt.float32, name="eps_t", tag="eps_t")
            nc.vector.memset(eps_t, EPS)
            me = small.tile([1, 1], mybir.dt.float32, name="me", tag="me")
            nc.scalar.activation(me, msum, _AFT.Identity, bias=eps_t[:, :], scale=1.0 / (2 * C))
            minv = small.tile([1, 1], mybir.dt.float32, name="minv", tag="minv")
            nc.vector.reciprocal(minv, me)
            mh = small.tile([1, 1], mybir.dt.float32, name="mh", tag="mh")
            nc.vector.tensor_scalar_mul(mh, minv, 0.5)

            # s = 1 + gamma * (r2 * 0.5 * minv)
            g_row = small.tile([1, C], mybir.dt.float32, name="g_row", tag="g_row")
            nc.sync.dma_start(out=g_row, in_=gamma[:])
            t2 = small.tile([1, C], mybir.dt.float32, name="t2", tag="t2")
            nc.vector.tensor_mul(t2, r2, g_row)
            s_row = small.tile([1, C], mybir.dt.float32, name="s_row", tag="s_row")
            nc.vector.tensor_scalar(
                s_row, t2, scalar1=mh[:, :], scalar2=1.0, op0=_ALU.mult, op1=_ALU.add
            )

            # broadcast s [1,64] -> [128,64] via K=1 matmul, then materialize
            # the contiguous [128, F] repeat straight from PSUM: pass-2 ops
            # then run on flat 2D operands instead of stride-0 broadcast APs
            # (which DVE executes ~1.5x slower and Pool ~1.4x slower)
            s_ps = psum.tile([P, C], mybir.dt.float32, name="s_ps", tag="s_ps")
            nc.tensor.matmul(s_ps[:, :], lhsT=ones_row[:, :], rhs=s_row[:, :], start=True, stop=True)
            s_rep = small.tile([P, K, C], mybir.dt.float16, name="s_rep", tag="s_rep")
            nc.vector.tensor_copy(s_rep, bcast_ap(s_ps, K))
            s_rep2 = s_rep.rearrange("p k c -> p (k c)")

            # --- pass 2: y = xh*s + beta, store ---------------------------
            # in-place fp16 mult on the (now dead) resident tile.  Then
            # either (cast_store) an in-place fp16 +beta and a SWDGE
            # cast-store straight from the resident tile, or a +beta that
            # upconverts into an f32 staging tile stored via HWDGE.  Adds
            # split DVE/Pool (Pool is ~2x slower per op but otherwise idle).
            for t in range(nt):
                f_t = ks[t] * C
                xh = xh_tiles[t]
                nc.vector.tensor_mul(xh, xh, s_rep2[:, :f_t])
                eng = nc.vector if (t * dve_adds) % nt < dve_adds else adder
                if cast_store:
                    eng.tensor_tensor(xh, xh, b_rep2[:, :f_t], _ALU.add)
                    nc.gpsimd.dma_start(out=out_view(t), in_=xh)
                else:
                    y = outp.tile([P, F], mybir.dt.float32, name="y", tag="y")[:, :f_t]
                    eng.tensor_tensor(y, xh, b_rep2[:, :f_t], _ALU.add)
                    seng = nc.scalar if t % 2 == 0 else nc.sync
                    seng.dma_start(out=out_view(t), in_=y)

    nc.finalize()
    return nc


def kernel(feat: np.ndarray, offset: np.ndarray, gamma: np.ndarray, beta: np.ndarray) -> np.ndarray:
    feat = np.ascontiguousarray(np.asarray(feat, dtype=np.float32))
    offset = np.asarray(offset)
    gamma = np.ascontiguousarray(np.asarray(gamma, dtype=np.float32)).reshape(1, C)
    beta = np.ascontiguousarray(np.asarray(beta, dtype=np.float32)).reshape(1, C)

    n = feat.shape[0]
    b = offset.shape[0]
    assert b <= N_CORES, f"need <= {N_CORES} segments, got {b}"

    ends = offset.astype(np.int64)
    starts = np.concatenate([[0], ends[:-1]])
    seg_rows = (ends - starts).astype(np.int64)

    r_max = int(seg_rows.max()) if b else P
    r_pad = max(P, ((r_max + P - 1) // P) * P)

    key = (r_pad,)
    nc = _program_cache.get(key)
    if nc is None:
        nc = _build_program(r_pad, **CONFIG)
        _program_cache[key] = nc

    shard_dt = np.float16 if CONFIG.get("host_fp16") else np.float32
    in_maps = []
    for i in range(N_CORES):
        shard = np.zeros((r_pad, C), dtype=shard_dt)
        if i < b and seg_rows[i] > 0:
            shard[: seg_rows[i]] = feat[starts[i] : ends[i]]
        in_maps.append({"feat": shard, "gamma": gamma, "beta": beta})

    results = run_bass_kernel_spmd(nc, in_maps, core_ids=list(range(N_CORES))).results

    out_full = np.empty((n, C), dtype=np.float32)
    for i in range(b):
        if seg_rows[i] > 0:
            out_full[starts[i] : ends[i]] = results[i]["out"][: seg_rows[i]]

    # Rows past offset[-1] (possible with general sorted offsets): the
    # reference's searchsorted yields index b there, which jax clamps to
    # b-1 on gather — those rows are scaled by the last segment's rn but
    # excluded from its sumsq.  Replicate on host.
    tail0 = int(ends[-1]) if b else 0
    if tail0 < n:
        last0, last1 = int(starts[-1]), int(ends[-1])
        sumsq = (feat[last0:last1].astype(np.float64) ** 2).sum(axis=0)
        r = np.sqrt(sumsq)
        rn = (r / (r.mean() + EPS)).astype(np.float32)
        ft = feat[tail0:]
        out_full[tail0:] = ft + gamma * (ft * rn[None, :]) + beta
    return out_full
